# revision 36
# baseline (speedup 1.0000x reference)
"""Trainium2 Bass kernel for the Tucker-factorized (TLE) multi-head attention.

Strategy (v2)
-------------
Data-parallel over batch: 16 batches / 8 cores = 2 batches per core; every
core runs the full per-batch pipeline (no collectives needed).

Host-side prep: the three per-mode factor matrices of each projection are
folded into one dense 768x768 Kronecker matrix.  Rows (for q/k/v) are
permuted to *head-major* order (h1,h2,h3,x,y,z) so each of the 12 heads
occupies a contiguous 64-partition block -- two heads per 128-partition
chunk.  The softmax scale 1/8 is folded into the q matrix/bias.  The o
matrix gets the inverse permutation on its rows.

v2 changes vs v1 (394us baseline):
  * Weights + xT + vT/vn/pp/yT in bfloat16: FWL (fast weight load) halves
    LDWEIGHTS on the projection matmuls, transposes run 1 cyc/row, DVE
    copies hit the 2x 16-bit mode, and PE power drops (the v1 trace showed
    the PE SW-throttled to 1.2 GHz for 53% of the run).
  * q/k + scores stay float32r for exp-input precision.
  * Score matmuls for the two heads of a chunk are issued adjacently: head0
    uses PE rows 0-63, head1 rows 64-127 (tile_position auto-derived from
    base_partition) so the two matmuls run concurrently in the array.
  * exp is batched over both heads (one ACT instruction per (sh, t5) pair
    reading both score banks) to amortize ScalarE access latency.
  * q/k bias adds moved to ScalarE (activation Identity+bias); softmax
    reciprocal reads PSUM directly (no staging copy).
  * Output projection computed token-major (out = yT.T @ Wo) which kills
    the final transpose pass entirely; bias added via a partition-broadcast
    bias tile on DVE.
"""

import numpy as np
import ml_dtypes

import concourse.bass as bass
import concourse.tile as tile
from concourse import bacc, mybir
from concourse.bass_utils import run_bass_kernel_spmd
from concourse.masks import make_identity

# ---------------------------------------------------------------- constants
N_CORES = 8
B = 16
BPC = B // N_CORES          # batches per core
P1, P2 = 25, 24
S = P1 * P2                 # 600 tokens
F = 768                     # flattened feature dim
FC = 6                      # feature chunks of 128
ST = 120                    # token tile
NS = S // ST                # 5 token tiles
NH = 300                    # half of the token axis
H1, H2, H3 = 2, 2, 3
XD = YD = ZD = 4
NHEADS = H1 * H2 * H3       # 12
HD = 64
F32 = mybir.dt.float32
BF16 = mybir.dt.bfloat16
F32R = mybir.dt.float32r
FO2 = F // 2                # 384, half of output-feature axis

import os
KDEBUG = os.environ.get("KDEBUG") == "1"
LDW_DEDUP = os.environ.get("LDW_DEDUP", "0") == "1"
QK_DT = BF16 if os.environ.get("QK_BF16", "0") == "1" else F32R


# ---------------------------------------------------------------- device IR
def _build_nc():
    nc = bacc.Bacc("TRN2", target_bir_lowering=False, debug=False)
    xr = nc.declare_dram_parameter("x", [BPC, S, F], BF16, isOutput=False)
    ws = [nc.declare_dram_parameter(f"w{m}", [FC, 128, FC, 128], BF16, isOutput=False)
          for m in range(4)]
    bs = [nc.declare_dram_parameter(f"b{m}", [F], F32, isOutput=False)
          for m in range(4)]
    outr = nc.declare_dram_parameter("out", [BPC, S, F], BF16, isOutput=True)
    dbg = {}
    if KDEBUG:
        for nm in ("xT", "vT", "yT"):
            dbg[nm] = nc.declare_dram_parameter(f"dbg_{nm}", [128, FC, S], BF16,
                                                isOutput=True)
        for nm in ("qT", "kT"):
            dbg[nm] = nc.declare_dram_parameter(
                f"dbg_{nm}", [128, FC, S],
                F32 if QK_DT == F32R else QK_DT, isOutput=True)
        dbg["vn"] = nc.declare_dram_parameter("dbg_vn", [128, NS, 2, HD + 1], BF16,
                                              isOutput=True)
        dbg["pp"] = nc.declare_dram_parameter("dbg_pp", [128, NS, 2, NH], BF16,
                                              isOutput=True)
        dbg["on"] = nc.declare_dram_parameter("dbg_on", [128, F], BF16,
                                              isOutput=True)
        dbg["py"] = nc.declare_dram_parameter("dbg_py", [128, 2, 512], F32,
                                              isOutput=True)
        dbg["rec2"] = nc.declare_dram_parameter("dbg_rec2", [1, 2, NH], F32,
                                                isOutput=True)
        dbg["rb"] = nc.declare_dram_parameter("dbg_rb", [HD, NH], F32,
                                              isOutput=True)

    def _dump(nm, ap):
        if KDEBUG:
            if ap.dtype == F32R:
                ap = ap.bitcast(F32)
            nc.sync.dma_start(out=dbg[nm][:], in_=ap)

    with tile.TileContext(nc) as tc:
        from contextlib import ExitStack
        with ExitStack() as ctx:
            const = ctx.enter_context(tc.tile_pool(name="const", bufs=1))
            big = ctx.enter_context(tc.tile_pool(name="big", bufs=2))
            qkp = ctx.enter_context(tc.tile_pool(name="qkp", bufs=3))
            vp = ctx.enter_context(tc.tile_pool(name="vp", bufs=2))
            stage = ctx.enter_context(tc.tile_pool(name="stage", bufs=2))
            vnp = ctx.enter_context(tc.tile_pool(name="vnp", bufs=2))
            ppp = ctx.enter_context(tc.tile_pool(name="ppp", bufs=2))
            recp = ctx.enter_context(tc.tile_pool(name="recp", bufs=2))
            # PSUM: 2 tags x 2 bufs x 2 banks = 8 banks total
            pwork = ctx.enter_context(tc.tile_pool(name="pwork", bufs=2, space="PSUM"))
            pscore = ctx.enter_context(tc.tile_pool(name="pscore", bufs=2, space="PSUM"))

            ident = const.tile([128, 128], F32, tag="ident")
            make_identity(nc, ident[:])
            ident_b = const.tile([128, 128], BF16, tag="identb")
            nc.vector.tensor_copy(ident_b[:], ident[:])
            ident_r = const.tile([128, 128], F32R, tag="identr")
            nc.vector.tensor_copy(ident_r[:], ident[:])

            # weights [128, 6(fi-chunk), 768(fo)] bf16 and biases.
            # Loaded in per-head-pair column slices, q/k/v slices first, on
            # two DMA queues, so pair-0 attention starts early.
            wsb = []
            for m in range(4):
                w = const.tile([128, FC, F], BF16, tag=f"w{m}")
                wsb.append(w)
            bsb = []
            for m in range(3):
                b = const.tile([128, FC], F32, tag=f"b{m}")
                nc.gpsimd.dma_start(out=b[:], in_=bs[m].rearrange("(c p) -> p c", p=128))
                bsb.append(b)
            # output bias, broadcast along partitions (free axis = fo)
            bo_row = const.tile([1, F], F32, tag="bo_row")
            nc.gpsimd.dma_start(out=bo_row[:, :], in_=bs[3].rearrange("(p f) -> p f", p=1))
            bias_o = const.tile([128, F], F32, tag="bias_o")
            nc.gpsimd.partition_broadcast(bias_o[:, :], bo_row[0:1, :])

            dma_engs = [nc.sync, nc.gpsimd]

            def load_xT(b):
                xT = big.tile([128, FC, S], BF16, tag="xT", name="xT")
                for st in range(NS):
                    xn = stage.tile([128, F], BF16, tag="xn", name="xn")
                    nc.scalar.dma_start(out=xn[:ST, :], in_=xr[b, st * ST:(st + 1) * ST, :])
                    slot = pwork.tile([128, 2, 512], F32, tag="proj", name="slot")
                    ptb = slot.bitcast(BF16)  # [128, 2, 1024]
                    for c in range(FC):
                        nc.tensor.transpose(
                            ptb[:, 0, c * ST:(c + 1) * ST],
                            xn[:ST, c * 128:(c + 1) * 128],
                            ident_b[:ST, :ST])
                    nc.vector.tensor_copy(
                        xT[:, :, st * ST:(st + 1) * ST],
                        ptb[:, 0, :FC * ST].rearrange("p (c s) -> p c s", c=FC))
                return xT

            # first head-pair q/k/v weight slices, then batch-0 x, then the rest
            qcount = 0
            for m in range(3):
                eng = dma_engs[qcount % 2]
                qcount += 1
                eng.dma_start(out=wsb[m][:, :, 0:128], in_=ws[m][0])
            xT0 = load_xT(0)
            for co in range(FC):
                for m in range(3):
                    if co == 0:
                        continue
                    eng = dma_engs[qcount % 2]
                    qcount += 1
                    eng.dma_start(
                        out=wsb[m][:, :, co * 128:(co + 1) * 128],
                        in_=ws[m][co])
            for co in range(FC):
                eng = dma_engs[qcount % 2]
                qcount += 1
                eng.dma_start(
                    out=wsb[3][:, :, co * 128:(co + 1) * 128],
                    in_=ws[3][co])

            for b in range(BPC):
                # ---- load x + transpose to feature-major -------------------
                xT = xT0 if b == 0 else load_xT(b)

                if b == 0:
                    _dump("xT", xT[:])

                # ---- fused per-pair projections + attention ---------------
                qT = qkp.tile([128, FC, S], QK_DT, tag="qkT")
                kT = qkp.tile([128, FC, S], QK_DT, tag="qkT")
                vT = vp.tile([128, FC, S], BF16, tag="vT")
                yT = big.tile([128, FC, S], BF16, tag="yT")
                def proj_chunk(m, hp, acc):
                    # 600-token projection of one 128-feature chunk; the h=1
                    # matmul reuses the h=0 stationary weights (no reload).
                    for ci in range(FC):
                        for h in range(2):
                            i = nc.tensor.matmul(
                                acc[:, h, :NH],
                                wsb[m][:, ci, hp * 128:(hp + 1) * 128],
                                xT[:, ci, h * NH:(h + 1) * NH],
                                start=(ci == 0), stop=(ci == FC - 1))
                            if h == 1 and LDW_DEDUP:
                                i.ins.ldweights = False

                for hp in range(FC):
                    # q, k projections (bias add on DVE, f32r out)
                    for m in range(2):
                        dst = (qT, kT)[m]
                        acc = pwork.tile([128, 2, 512], F32, tag="proj")
                        proj_chunk(m, hp, acc)
                        nc.vector.tensor_scalar_add(
                            dst[:, hp, 0:NH],
                            in0=acc[:, 0, :NH],
                            scalar1=bsb[m][:, hp:hp + 1])
                        nc.scalar.add(
                            dst[:, hp, NH:2 * NH],
                            acc[:, 1, :NH],
                            add=bsb[m][:, hp:hp + 1])
                    # v projection (bias add on DVE, bf16 out)
                    accv = pwork.tile([128, 2, 512], F32, tag="proj")
                    proj_chunk(2, hp, accv)
                    nc.vector.tensor_scalar_add(
                        vT[:, hp, 0:NH],
                        in0=accv[:, 0, :NH],
                        scalar1=bsb[2][:, hp:hp + 1])
                    nc.scalar.add(
                        vT[:, hp, NH:2 * NH],
                        accv[:, 1, :NH],
                        add=bsb[2][:, hp:hp + 1])

                    # V back to token-major with an appended ones column
                    vn = vnp.tile([128, NS, 2, HD + 1], BF16, tag="vn")
                    nc.gpsimd.memset(vn[:ST, :, :, HD:HD + 1], 1.0)
                    vslot = pwork.tile([128, 2, 512], F32, tag="proj")
                    pv_b = vslot.bitcast(BF16)  # [128, 2, 1024]
                    for t5 in range(NS):
                        nc.tensor.transpose(
                            pv_b[:ST, 0, t5 * 128:(t5 + 1) * 128],
                            vT[:, hp, t5 * ST:(t5 + 1) * ST],
                            ident_b[:, :])
                    nc.vector.tensor_copy(
                        vn[:ST, :, :, 0:HD],
                        pv_b[:ST, 0, :NS * 128].rearrange("p (t g d) -> p t g d",
                                                          t=NS, g=2))
                    if b == 0 and hp == 0:
                        _dump("vn", vn[:])

                    for sh in range(2):
                        pp = ppp.tile([128, NS, 2, NH], BF16, tag="pp")
                        py = pwork.tile([128, 2, 512], F32, tag="proj")
                        for t5 in range(NS):
                            sc = pscore.tile([128, 2, 512], F32, tag="score")
                            for g in range(2):
                                r0 = g * HD
                                nc.tensor.matmul(
                                    sc[:ST, g, :NH],
                                    kT[r0:r0 + HD, hp, t5 * ST:(t5 + 1) * ST],
                                    qT[r0:r0 + HD, hp, sh * NH:(sh + 1) * NH],
                                    start=True, stop=True)
                            nc.scalar.activation(
                                pp[:ST, t5, :, :], sc[:ST, :, :NH],
                                func=mybir.ActivationFunctionType.Exp)
                        for g in range(2):
                            for t5 in range(NS):
                                nc.tensor.matmul(
                                    py[:HD + 1, g, :NH],
                                    vn[:ST, t5, g, :],
                                    pp[:ST, t5, g, :],
                                    start=(t5 == 0), stop=(t5 == NS - 1))
                        if b == 0 and hp == 0 and sh == 0:
                            _dump("pp", pp[:])
                            if KDEBUG:
                                pyc = recp.tile([128, 2, 512], F32, tag="pyc",
                                                bufs=1)
                                nc.vector.tensor_copy(pyc[:], py[:])
                                _dump("py", pyc[:])
                        # softmax normalize: ones-column sums live in
                        # partition HD of each bank
                        for g in range(2):
                            srow = recp.tile([1, NH], F32, tag="srow")
                            nc.vector.tensor_copy(srow[:, :], py[HD:HD + 1, g, :NH])
                            rec = recp.tile([1, NH], F32, tag="rec")
                            nc.vector.reciprocal_approx_fast(rec[:, :], srow[:, :])
                            rb = recp.tile([HD, NH], F32, tag="rb")
                            nc.gpsimd.partition_broadcast(rb[:, :], rec[0:1, :])
                            nc.vector.tensor_mul(
                                yT[g * HD:(g + 1) * HD, hp, sh * NH:(sh + 1) * NH],
                                py[:HD, g, :NH], rb[:, :])
                if b == 0:
                    _dump("qT", qT[:].bitcast(F32) if QK_DT == F32R else qT[:])
                    _dump("kT", kT[:].bitcast(F32) if QK_DT == F32R else kT[:])
                    _dump("vT", vT[:])
                    _dump("yT", yT[:])

                # ---- output projection, token-major -----------------------
                for tb in range(NS):
                    oslot = pwork.tile([128, 2, 512], F32, tag="proj")
                    for ci in range(FC):
                        for half in range(2):
                            i = nc.tensor.matmul(
                                oslot[:ST, half, :FO2],
                                yT[:, ci, tb * ST:(tb + 1) * ST],
                                wsb[3][:, ci, half * FO2:(half + 1) * FO2],
                                start=(ci == 0), stop=(ci == FC - 1))
                            if half == 1 and LDW_DEDUP:
                                i.ins.ldweights = False
                    on = stage.tile([128, F], BF16, tag="on")
                    nc.vector.tensor_add(
                        on[:ST, :].rearrange("p (h n) -> p h n", h=2),
                        oslot[:ST, :, :FO2],
                        bias_o[:ST, :].rearrange("p (h n) -> p h n", h=2))
                    if b == 0 and tb == 0:
                        _dump("on", on[:])
                    nc.sync.dma_start(out=outr[b, tb * ST:(tb + 1) * ST, :],
                                      in_=on[:ST, :])

    nc.finalize()
    return nc


_NC_CACHE = {}


def _get_nc():
    if "nc" not in _NC_CACHE:
        _NC_CACHE["nc"] = _build_nc()
    return _NC_CACHE["nc"]


# ------------------------------------------------------------- host wrapper
def _head_major_perm():
    perm = np.empty(F, dtype=np.int64)
    i = 0
    for h1 in range(H1):
        for h2 in range(H2):
            for h3 in range(H3):
                for x in range(XD):
                    for y in range(YD):
                        for z in range(ZD):
                            a = x * H1 + h1
                            bb = y * H2 + h2
                            cc = z * H3 + h3
                            perm[i] = a * 96 + bb * 12 + cc
                            i += 1
    return perm


def _prep_inputs(inputs):
    perm = _head_major_perm()
    scale = float(HD) ** -0.5

    def kron3(w1, w2, w3):
        return np.kron(w1, np.kron(w2, w3)).astype(np.float32)

    mats = {}
    mats["w0"] = np.ascontiguousarray(
        (kron3(inputs["Wq1"], inputs["Wq2"], inputs["Wq3"])[perm, :] * scale).T)
    mats["b0"] = np.ascontiguousarray(
        inputs["bq"].reshape(F)[perm] * scale).astype(np.float32)
    mats["w1"] = np.ascontiguousarray(
        kron3(inputs["Wk1"], inputs["Wk2"], inputs["Wk3"])[perm, :].T)
    mats["b1"] = np.ascontiguousarray(inputs["bk"].reshape(F)[perm]).astype(np.float32)
    mats["w2"] = np.ascontiguousarray(
        kron3(inputs["Wv1"], inputs["Wv2"], inputs["Wv3"])[perm, :].T)
    mats["b2"] = np.ascontiguousarray(inputs["bv"].reshape(F)[perm]).astype(np.float32)
    mats["w3"] = np.ascontiguousarray(
        kron3(inputs["Wo1"], inputs["Wo2"], inputs["Wo3"])[:, perm].T)
    mats["b3"] = np.ascontiguousarray(inputs["bo"].reshape(F)).astype(np.float32)
    return mats


def _make_in_maps(inputs):
    mats = _prep_inputs(inputs)
    for k in ("w0", "w1", "w2", "w3"):
        mats[k] = np.ascontiguousarray(
            mats[k].reshape(FC, 128, FC, 128).transpose(2, 1, 0, 3)
        ).astype(ml_dtypes.bfloat16)
    x = np.asarray(inputs["x"], dtype=np.float32).reshape(B, S, F)
    x = np.ascontiguousarray(x).astype(ml_dtypes.bfloat16)
    in_maps = []
    for c in range(N_CORES):
        m = {"x": np.ascontiguousarray(x[c * BPC:(c + 1) * BPC])}
        m.update(mats)
        in_maps.append(m)
    return in_maps


def kernel(**inputs) -> np.ndarray:
    nc = _get_nc()
    in_maps = _make_in_maps(inputs)
    res = run_bass_kernel_spmd(nc, in_maps, core_ids=list(range(N_CORES)))
    out = np.concatenate([res.results[c]["out"] for c in range(N_CORES)], axis=0)
    return out.reshape(B, P1, P2, 8, 8, 12).astype(np.float32)


def run_traced(inputs, **kw):
    """test.py helper: returns (output, BassKernelResults) with trace."""
    nc = _get_nc()
    in_maps = _make_in_maps(inputs)
    res = run_bass_kernel_spmd(nc, in_maps, core_ids=list(range(N_CORES)), **kw)
    out = np.concatenate([res.results[c]["out"] for c in range(N_CORES)], axis=0)
    return out.reshape(B, P1, P2, 8, 8, 12).astype(np.float32), res


# revision 39
# speedup vs baseline: 1.0260x; 1.0260x over previous
"""Trainium2 Bass kernel for the Tucker-factorized (TLE) multi-head attention.

Strategy (v2)
-------------
Data-parallel over batch: 16 batches / 8 cores = 2 batches per core; every
core runs the full per-batch pipeline (no collectives needed).

Host-side prep: the three per-mode factor matrices of each projection are
folded into one dense 768x768 Kronecker matrix.  Rows (for q/k/v) are
permuted to *head-major* order (h1,h2,h3,x,y,z) so each of the 12 heads
occupies a contiguous 64-partition block -- two heads per 128-partition
chunk.  The softmax scale 1/8 is folded into the q matrix/bias.  The o
matrix gets the inverse permutation on its rows.

v2 changes vs v1 (394us baseline):
  * Weights + xT + vT/vn/pp/yT in bfloat16: FWL (fast weight load) halves
    LDWEIGHTS on the projection matmuls, transposes run 1 cyc/row, DVE
    copies hit the 2x 16-bit mode, and PE power drops (the v1 trace showed
    the PE SW-throttled to 1.2 GHz for 53% of the run).
  * q/k + scores stay float32r for exp-input precision.
  * Score matmuls for the two heads of a chunk are issued adjacently: head0
    uses PE rows 0-63, head1 rows 64-127 (tile_position auto-derived from
    base_partition) so the two matmuls run concurrently in the array.
  * exp is batched over both heads (one ACT instruction per (sh, t5) pair
    reading both score banks) to amortize ScalarE access latency.
  * q/k bias adds moved to ScalarE (activation Identity+bias); softmax
    reciprocal reads PSUM directly (no staging copy).
  * Output projection computed token-major (out = yT.T @ Wo) which kills
    the final transpose pass entirely; bias added via a partition-broadcast
    bias tile on DVE.
"""

import numpy as np
import ml_dtypes

import concourse.bass as bass
import concourse.tile as tile
from concourse import bacc, mybir
from concourse.bass_utils import run_bass_kernel_spmd
from concourse.masks import make_identity

# ---------------------------------------------------------------- constants
N_CORES = 8
B = 16
BPC = B // N_CORES          # batches per core
P1, P2 = 25, 24
S = P1 * P2                 # 600 tokens
F = 768                     # flattened feature dim
FC = 6                      # feature chunks of 128
ST = 120                    # token tile
NS = S // ST                # 5 token tiles
NH = 300                    # half of the token axis
H1, H2, H3 = 2, 2, 3
XD = YD = ZD = 4
NHEADS = H1 * H2 * H3       # 12
HD = 64
F32 = mybir.dt.float32
BF16 = mybir.dt.bfloat16
F32R = mybir.dt.float32r
FO2 = F // 2                # 384, half of output-feature axis

import os
KDEBUG = os.environ.get("KDEBUG") == "1"
LDW_DEDUP = os.environ.get("LDW_DEDUP", "0") == "1"
QK_DT = BF16 if os.environ.get("QK_BF16", "0") == "1" else F32R


# ---------------------------------------------------------------- device IR
def _build_nc():
    nc = bacc.Bacc("TRN2", target_bir_lowering=False, debug=False)
    xr = nc.declare_dram_parameter("x", [BPC, S, F], BF16, isOutput=False)
    ws = [nc.declare_dram_parameter(f"w{m}", [FC, 128, FC, 128], BF16, isOutput=False)
          for m in range(4)]
    bs = [nc.declare_dram_parameter(f"b{m}", [F], F32, isOutput=False)
          for m in range(4)]
    outr = nc.declare_dram_parameter("out", [BPC, S, F], BF16, isOutput=True)
    dbg = {}
    if KDEBUG:
        for nm in ("xT", "vT", "yT"):
            dbg[nm] = nc.declare_dram_parameter(f"dbg_{nm}", [128, FC, S], BF16,
                                                isOutput=True)
        for nm in ("qT", "kT"):
            dbg[nm] = nc.declare_dram_parameter(
                f"dbg_{nm}", [128, FC, S],
                F32 if QK_DT == F32R else QK_DT, isOutput=True)
        dbg["vn"] = nc.declare_dram_parameter("dbg_vn", [128, NS, 2, HD + 1], BF16,
                                              isOutput=True)
        dbg["pp"] = nc.declare_dram_parameter("dbg_pp", [128, NS, 2, NH], BF16,
                                              isOutput=True)
        dbg["on"] = nc.declare_dram_parameter("dbg_on", [128, F], BF16,
                                              isOutput=True)
        dbg["py"] = nc.declare_dram_parameter("dbg_py", [128, 2, 512], F32,
                                              isOutput=True)
        dbg["rec2"] = nc.declare_dram_parameter("dbg_rec2", [1, 2, NH], F32,
                                                isOutput=True)
        dbg["rb"] = nc.declare_dram_parameter("dbg_rb", [HD, NH], F32,
                                              isOutput=True)

    def _dump(nm, ap):
        if KDEBUG:
            if ap.dtype == F32R:
                ap = ap.bitcast(F32)
            nc.sync.dma_start(out=dbg[nm][:], in_=ap)

    with tile.TileContext(nc) as tc:
        from contextlib import ExitStack
        with ExitStack() as ctx:
            const = ctx.enter_context(tc.tile_pool(name="const", bufs=1))
            big = ctx.enter_context(tc.tile_pool(name="big", bufs=2))
            qkp = ctx.enter_context(tc.tile_pool(name="qkp", bufs=3))
            vp = ctx.enter_context(tc.tile_pool(name="vp", bufs=2))
            stage = ctx.enter_context(tc.tile_pool(name="stage", bufs=2))
            vnp = ctx.enter_context(tc.tile_pool(name="vnp", bufs=2))
            ppp = ctx.enter_context(tc.tile_pool(name="ppp", bufs=2))
            recp = ctx.enter_context(tc.tile_pool(name="recp", bufs=2))
            # PSUM: 2 tags x 2 bufs x 2 banks = 8 banks total
            pwork = ctx.enter_context(tc.tile_pool(name="pwork", bufs=2, space="PSUM"))
            pscore = ctx.enter_context(tc.tile_pool(name="pscore", bufs=2, space="PSUM"))

            ident = const.tile([128, 128], F32, tag="ident")
            make_identity(nc, ident[:])
            ident_b = const.tile([128, 128], BF16, tag="identb")
            nc.vector.tensor_copy(ident_b[:], ident[:])
            ident_r = const.tile([128, 128], F32R, tag="identr")
            nc.vector.tensor_copy(ident_r[:], ident[:])

            # weights [128, 6(fi-chunk), 768(fo)] bf16 and biases.
            # Loaded in per-head-pair column slices, q/k/v slices first, on
            # two DMA queues, so pair-0 attention starts early.
            wsb = []
            for m in range(4):
                w = const.tile([128, FC, F], BF16, tag=f"w{m}")
                wsb.append(w)
            bsb = []
            for m in range(3):
                b = const.tile([128, FC], F32, tag=f"b{m}")
                nc.gpsimd.dma_start(out=b[:], in_=bs[m].rearrange("(c p) -> p c", p=128))
                bsb.append(b)
            # output bias, broadcast along partitions (free axis = fo)
            bo_row = const.tile([1, F], F32, tag="bo_row")
            nc.gpsimd.dma_start(out=bo_row[:, :], in_=bs[3].rearrange("(p f) -> p f", p=1))
            bias_o = const.tile([128, F], F32, tag="bias_o")
            nc.gpsimd.partition_broadcast(bias_o[:, :], bo_row[0:1, :])

            dma_engs = [nc.sync, nc.gpsimd, nc.scalar]

            def load_xT(b):
                xT = big.tile([128, FC, S], BF16, tag="xT", name="xT")
                for st in range(NS):
                    xn = stage.tile([128, F], BF16, tag="xn", name="xn")
                    nc.scalar.dma_start(out=xn[:ST, :], in_=xr[b, st * ST:(st + 1) * ST, :])
                    slot = pwork.tile([128, 2, 512], F32, tag="proj", name="slot")
                    ptb = slot.bitcast(BF16)  # [128, 2, 1024]
                    for c in range(FC):
                        nc.tensor.transpose(
                            ptb[:, 0, c * ST:(c + 1) * ST],
                            xn[:ST, c * 128:(c + 1) * 128],
                            ident_b[:ST, :ST])
                    nc.vector.tensor_copy(
                        xT[:, :, st * ST:(st + 1) * ST],
                        ptb[:, 0, :FC * ST].rearrange("p (c s) -> p c s", c=FC))
                return xT

            # first head-pair q/k/v weight slices, then batch-0 x, then the rest
            qcount = 0
            for m in range(3):
                eng = dma_engs[qcount % 3]
                qcount += 1
                eng.dma_start(out=wsb[m][:, :, 0:128], in_=ws[m][0])
            xT0 = load_xT(0)
            for co in range(FC):
                for m in range(3):
                    if co == 0:
                        continue
                    eng = dma_engs[qcount % 3]
                    qcount += 1
                    eng.dma_start(
                        out=wsb[m][:, :, co * 128:(co + 1) * 128],
                        in_=ws[m][co])
            for co in range(FC):
                eng = dma_engs[qcount % 3]
                qcount += 1
                eng.dma_start(
                    out=wsb[3][:, :, co * 128:(co + 1) * 128],
                    in_=ws[3][co])

            for b in range(BPC):
                # ---- load x + transpose to feature-major -------------------
                xT = xT0 if b == 0 else load_xT(b)

                if b == 0:
                    _dump("xT", xT[:])

                # ---- fused per-pair projections + attention ---------------
                qT = qkp.tile([128, FC, S], QK_DT, tag="qkT")
                kT = qkp.tile([128, FC, S], QK_DT, tag="qkT")
                vT = vp.tile([128, FC, S], BF16, tag="vT")
                yT = big.tile([128, FC, S], BF16, tag="yT")
                def proj_chunk(m, hp, acc):
                    # 600-token projection of one 128-feature chunk; the h=1
                    # matmul reuses the h=0 stationary weights (no reload).
                    for ci in range(FC):
                        for h in range(2):
                            i = nc.tensor.matmul(
                                acc[:, h, :NH],
                                wsb[m][:, ci, hp * 128:(hp + 1) * 128],
                                xT[:, ci, h * NH:(h + 1) * NH],
                                start=(ci == 0), stop=(ci == FC - 1))
                            if h == 1 and LDW_DEDUP:
                                i.ins.ldweights = False

                for hp in range(FC):
                    # q, k projections (bias add on DVE, f32r out)
                    for m in range(2):
                        dst = (qT, kT)[m]
                        acc = pwork.tile([128, 2, 512], F32, tag="proj")
                        proj_chunk(m, hp, acc)
                        nc.scalar.add(
                            dst[:, hp, :].rearrange("p (h n) -> p h n", h=2),
                            acc[:, :, :NH],
                            add=bsb[m][:, hp:hp + 1])
                    # v projection (bias add on DVE, bf16 out)
                    accv = pwork.tile([128, 2, 512], F32, tag="proj")
                    proj_chunk(2, hp, accv)
                    nc.vector.tensor_scalar_add(
                        vT[:, hp, :].rearrange("p (h n) -> p h n", h=2),
                        in0=accv[:, :, :NH],
                        scalar1=bsb[2][:, hp:hp + 1])

                    # V back to token-major with an appended ones column
                    vn = vnp.tile([128, NS, 2, HD + 1], BF16, tag="vn")
                    nc.gpsimd.memset(vn[:ST, :, :, HD:HD + 1], 1.0)
                    vslot = pwork.tile([128, 2, 512], F32, tag="proj")
                    pv_b = vslot.bitcast(BF16)  # [128, 2, 1024]
                    for t5 in range(NS):
                        nc.tensor.transpose(
                            pv_b[:ST, 0, t5 * 128:(t5 + 1) * 128],
                            vT[:, hp, t5 * ST:(t5 + 1) * ST],
                            ident_b[:, :])
                    nc.vector.tensor_copy(
                        vn[:ST, :, :, 0:HD],
                        pv_b[:ST, 0, :NS * 128].rearrange("p (t g d) -> p t g d",
                                                          t=NS, g=2))
                    if b == 0 and hp == 0:
                        _dump("vn", vn[:])

                    for sh in range(2):
                        pp = ppp.tile([128, NS, 2, NH], BF16, tag="pp")
                        py = pwork.tile([128, 2, 512], F32, tag="proj")
                        for t5 in range(NS):
                            sc = pscore.tile([128, 2, 512], F32, tag="score")
                            for g in range(2):
                                r0 = g * HD
                                nc.tensor.matmul(
                                    sc[:ST, g, :NH],
                                    kT[r0:r0 + HD, hp, t5 * ST:(t5 + 1) * ST],
                                    qT[r0:r0 + HD, hp, sh * NH:(sh + 1) * NH],
                                    start=True, stop=True)
                            nc.scalar.activation(
                                pp[:ST, t5, :, :], sc[:ST, :, :NH],
                                func=mybir.ActivationFunctionType.Exp)
                        for g in range(2):
                            for t5 in range(NS):
                                nc.tensor.matmul(
                                    py[:HD + 1, g, :NH],
                                    vn[:ST, t5, g, :],
                                    pp[:ST, t5, g, :],
                                    start=(t5 == 0), stop=(t5 == NS - 1))
                        if b == 0 and hp == 0 and sh == 0:
                            _dump("pp", pp[:])
                            if KDEBUG:
                                pyc = recp.tile([128, 2, 512], F32, tag="pyc",
                                                bufs=1)
                                nc.vector.tensor_copy(pyc[:], py[:])
                                _dump("py", pyc[:])
                        # softmax normalize: ones-column sums live in
                        # partition HD of each bank
                        srow2 = recp.tile([1, 2, NH], F32, tag="srow2")
                        nc.vector.tensor_copy(srow2[:, :, :], py[HD:HD + 1, :, :NH])
                        rec2 = recp.tile([1, 2, NH], F32, tag="rec2")
                        nc.vector.reciprocal_approx_fast(
                            rec2[:, :, :], srow2[:, :, :])
                        for g in range(2):
                            rb = recp.tile([HD, NH], F32, tag="rb")
                            nc.gpsimd.partition_broadcast(rb[:, :], rec2[0:1, g, :])
                            nc.vector.tensor_mul(
                                yT[g * HD:(g + 1) * HD, hp, sh * NH:(sh + 1) * NH],
                                py[:HD, g, :NH], rb[:, :])
                if b == 0:
                    _dump("qT", qT[:].bitcast(F32) if QK_DT == F32R else qT[:])
                    _dump("kT", kT[:].bitcast(F32) if QK_DT == F32R else kT[:])
                    _dump("vT", vT[:])
                    _dump("yT", yT[:])

                # ---- output projection, token-major -----------------------
                for tb in range(NS):
                    oslot = pwork.tile([128, 2, 512], F32, tag="proj")
                    for ci in range(FC):
                        for half in range(2):
                            i = nc.tensor.matmul(
                                oslot[:ST, half, :FO2],
                                yT[:, ci, tb * ST:(tb + 1) * ST],
                                wsb[3][:, ci, half * FO2:(half + 1) * FO2],
                                start=(ci == 0), stop=(ci == FC - 1))
                            if half == 1 and LDW_DEDUP:
                                i.ins.ldweights = False
                    on = stage.tile([128, F], BF16, tag="on")
                    nc.vector.tensor_add(
                        on[:ST, :].rearrange("p (h n) -> p h n", h=2),
                        oslot[:ST, :, :FO2],
                        bias_o[:ST, :].rearrange("p (h n) -> p h n", h=2))
                    if b == 0 and tb == 0:
                        _dump("on", on[:])
                    nc.sync.dma_start(out=outr[b, tb * ST:(tb + 1) * ST, :],
                                      in_=on[:ST, :])

    nc.finalize()
    return nc


_NC_CACHE = {}


def _get_nc():
    if "nc" not in _NC_CACHE:
        _NC_CACHE["nc"] = _build_nc()
    return _NC_CACHE["nc"]


# ------------------------------------------------------------- host wrapper
def _head_major_perm():
    perm = np.empty(F, dtype=np.int64)
    i = 0
    for h1 in range(H1):
        for h2 in range(H2):
            for h3 in range(H3):
                for x in range(XD):
                    for y in range(YD):
                        for z in range(ZD):
                            a = x * H1 + h1
                            bb = y * H2 + h2
                            cc = z * H3 + h3
                            perm[i] = a * 96 + bb * 12 + cc
                            i += 1
    return perm


def _prep_inputs(inputs):
    perm = _head_major_perm()
    scale = float(HD) ** -0.5

    def kron3(w1, w2, w3):
        return np.kron(w1, np.kron(w2, w3)).astype(np.float32)

    mats = {}
    mats["w0"] = np.ascontiguousarray(
        (kron3(inputs["Wq1"], inputs["Wq2"], inputs["Wq3"])[perm, :] * scale).T)
    mats["b0"] = np.ascontiguousarray(
        inputs["bq"].reshape(F)[perm] * scale).astype(np.float32)
    mats["w1"] = np.ascontiguousarray(
        kron3(inputs["Wk1"], inputs["Wk2"], inputs["Wk3"])[perm, :].T)
    mats["b1"] = np.ascontiguousarray(inputs["bk"].reshape(F)[perm]).astype(np.float32)
    mats["w2"] = np.ascontiguousarray(
        kron3(inputs["Wv1"], inputs["Wv2"], inputs["Wv3"])[perm, :].T)
    mats["b2"] = np.ascontiguousarray(inputs["bv"].reshape(F)[perm]).astype(np.float32)
    mats["w3"] = np.ascontiguousarray(
        kron3(inputs["Wo1"], inputs["Wo2"], inputs["Wo3"])[:, perm].T)
    mats["b3"] = np.ascontiguousarray(inputs["bo"].reshape(F)).astype(np.float32)
    return mats


def _make_in_maps(inputs):
    mats = _prep_inputs(inputs)
    for k in ("w0", "w1", "w2", "w3"):
        mats[k] = np.ascontiguousarray(
            mats[k].reshape(FC, 128, FC, 128).transpose(2, 1, 0, 3)
        ).astype(ml_dtypes.bfloat16)
    x = np.asarray(inputs["x"], dtype=np.float32).reshape(B, S, F)
    x = np.ascontiguousarray(x).astype(ml_dtypes.bfloat16)
    in_maps = []
    for c in range(N_CORES):
        m = {"x": np.ascontiguousarray(x[c * BPC:(c + 1) * BPC])}
        m.update(mats)
        in_maps.append(m)
    return in_maps


def kernel(**inputs) -> np.ndarray:
    nc = _get_nc()
    in_maps = _make_in_maps(inputs)
    res = run_bass_kernel_spmd(nc, in_maps, core_ids=list(range(N_CORES)))
    out = np.concatenate([res.results[c]["out"] for c in range(N_CORES)], axis=0)
    return out.reshape(B, P1, P2, 8, 8, 12).astype(np.float32)


def run_traced(inputs, **kw):
    """test.py helper: returns (output, BassKernelResults) with trace."""
    nc = _get_nc()
    in_maps = _make_in_maps(inputs)
    res = run_bass_kernel_spmd(nc, in_maps, core_ids=list(range(N_CORES)), **kw)
    out = np.concatenate([res.results[c]["out"] for c in range(N_CORES)], axis=0)
    return out.reshape(B, P1, P2, 8, 8, 12).astype(np.float32), res


# revision 40
# speedup vs baseline: 1.0562x; 1.0295x over previous
"""Trainium2 Bass kernel for the Tucker-factorized (TLE) multi-head attention.

Strategy (v2)
-------------
Data-parallel over batch: 16 batches / 8 cores = 2 batches per core; every
core runs the full per-batch pipeline (no collectives needed).

Host-side prep: the three per-mode factor matrices of each projection are
folded into one dense 768x768 Kronecker matrix.  Rows (for q/k/v) are
permuted to *head-major* order (h1,h2,h3,x,y,z) so each of the 12 heads
occupies a contiguous 64-partition block -- two heads per 128-partition
chunk.  The softmax scale 1/8 is folded into the q matrix/bias.  The o
matrix gets the inverse permutation on its rows.

v2 changes vs v1 (394us baseline):
  * Weights + xT + vT/vn/pp/yT in bfloat16: FWL (fast weight load) halves
    LDWEIGHTS on the projection matmuls, transposes run 1 cyc/row, DVE
    copies hit the 2x 16-bit mode, and PE power drops (the v1 trace showed
    the PE SW-throttled to 1.2 GHz for 53% of the run).
  * q/k + scores stay float32r for exp-input precision.
  * Score matmuls for the two heads of a chunk are issued adjacently: head0
    uses PE rows 0-63, head1 rows 64-127 (tile_position auto-derived from
    base_partition) so the two matmuls run concurrently in the array.
  * exp is batched over both heads (one ACT instruction per (sh, t5) pair
    reading both score banks) to amortize ScalarE access latency.
  * q/k bias adds moved to ScalarE (activation Identity+bias); softmax
    reciprocal reads PSUM directly (no staging copy).
  * Output projection computed token-major (out = yT.T @ Wo) which kills
    the final transpose pass entirely; bias added via a partition-broadcast
    bias tile on DVE.
"""

import numpy as np
import ml_dtypes

import concourse.bass as bass
import concourse.tile as tile
from concourse import bacc, mybir
from concourse.bass_utils import run_bass_kernel_spmd
from concourse.masks import make_identity

# ---------------------------------------------------------------- constants
N_CORES = 8
B = 16
BPC = B // N_CORES          # batches per core
P1, P2 = 25, 24
S = P1 * P2                 # 600 tokens
F = 768                     # flattened feature dim
FC = 6                      # feature chunks of 128
ST = 120                    # token tile
NS = S // ST                # 5 token tiles
NH = 300                    # half of the token axis
H1, H2, H3 = 2, 2, 3
XD = YD = ZD = 4
NHEADS = H1 * H2 * H3       # 12
HD = 64
F32 = mybir.dt.float32
BF16 = mybir.dt.bfloat16
F32R = mybir.dt.float32r
FO2 = F // 2                # 384, half of output-feature axis

import os
KDEBUG = os.environ.get("KDEBUG") == "1"
LDW_DEDUP = os.environ.get("LDW_DEDUP", "0") == "1"
QK_DT = BF16 if os.environ.get("QK_BF16", "0") == "1" else F32R


# ---------------------------------------------------------------- device IR
def _build_nc():
    nc = bacc.Bacc("TRN2", target_bir_lowering=False, debug=False)
    xr = nc.declare_dram_parameter("x", [BPC, S, F], BF16, isOutput=False)
    ws = [nc.declare_dram_parameter(f"w{m}", [FC, 128, FC, 128], BF16, isOutput=False)
          for m in range(4)]
    bs = [nc.declare_dram_parameter(f"b{m}", [F], F32, isOutput=False)
          for m in range(4)]
    outr = nc.declare_dram_parameter("out", [BPC, S, F], BF16, isOutput=True)
    dbg = {}
    if KDEBUG:
        for nm in ("xT", "vT", "yT"):
            dbg[nm] = nc.declare_dram_parameter(f"dbg_{nm}", [128, FC, S], BF16,
                                                isOutput=True)
        for nm in ("qT", "kT"):
            dbg[nm] = nc.declare_dram_parameter(
                f"dbg_{nm}", [128, FC, S],
                F32 if QK_DT == F32R else QK_DT, isOutput=True)
        dbg["vn"] = nc.declare_dram_parameter("dbg_vn", [128, NS, 2, HD + 1], BF16,
                                              isOutput=True)
        dbg["pp"] = nc.declare_dram_parameter("dbg_pp", [128, NS, 2, NH], BF16,
                                              isOutput=True)
        dbg["on"] = nc.declare_dram_parameter("dbg_on", [128, F], BF16,
                                              isOutput=True)
        dbg["py"] = nc.declare_dram_parameter("dbg_py", [128, 2, 512], F32,
                                              isOutput=True)
        dbg["rec2"] = nc.declare_dram_parameter("dbg_rec2", [1, 2, NH], F32,
                                                isOutput=True)
        dbg["rb"] = nc.declare_dram_parameter("dbg_rb", [HD, NH], F32,
                                              isOutput=True)

    def _dump(nm, ap):
        if KDEBUG:
            if ap.dtype == F32R:
                ap = ap.bitcast(F32)
            nc.sync.dma_start(out=dbg[nm][:], in_=ap)

    with tile.TileContext(nc) as tc:
        from contextlib import ExitStack
        with ExitStack() as ctx:
            const = ctx.enter_context(tc.tile_pool(name="const", bufs=1))
            big = ctx.enter_context(tc.tile_pool(name="big", bufs=2))
            qkp = ctx.enter_context(tc.tile_pool(name="qkp", bufs=3))
            vp = ctx.enter_context(tc.tile_pool(name="vp", bufs=2))
            stage = ctx.enter_context(tc.tile_pool(name="stage", bufs=3))
            vnp = ctx.enter_context(tc.tile_pool(name="vnp", bufs=3))
            ppp = ctx.enter_context(tc.tile_pool(name="ppp", bufs=3))
            recp = ctx.enter_context(tc.tile_pool(name="recp", bufs=3))
            # PSUM: 2 tags x 2 bufs x 2 banks = 8 banks total
            pwork = ctx.enter_context(tc.tile_pool(name="pwork", bufs=2, space="PSUM"))
            pscore = ctx.enter_context(tc.tile_pool(name="pscore", bufs=2, space="PSUM"))

            ident = const.tile([128, 128], F32, tag="ident")
            make_identity(nc, ident[:])
            ident_b = const.tile([128, 128], BF16, tag="identb")
            nc.vector.tensor_copy(ident_b[:], ident[:])
            ident_r = const.tile([128, 128], F32R, tag="identr")
            nc.vector.tensor_copy(ident_r[:], ident[:])

            # weights [128, 6(fi-chunk), 768(fo)] bf16 and biases.
            # Loaded in per-head-pair column slices, q/k/v slices first, on
            # two DMA queues, so pair-0 attention starts early.
            wsb = []
            for m in range(4):
                w = const.tile([128, FC, F], BF16, tag=f"w{m}")
                wsb.append(w)
            bsb = []
            for m in range(3):
                b = const.tile([128, FC], F32, tag=f"b{m}")
                nc.gpsimd.dma_start(out=b[:], in_=bs[m].rearrange("(c p) -> p c", p=128))
                bsb.append(b)
            # output bias, broadcast along partitions (free axis = fo)
            bo_row = const.tile([1, F], F32, tag="bo_row")
            nc.gpsimd.dma_start(out=bo_row[:, :], in_=bs[3].rearrange("(p f) -> p f", p=1))
            bias_o = const.tile([128, F], F32, tag="bias_o")
            nc.gpsimd.partition_broadcast(bias_o[:, :], bo_row[0:1, :])

            dma_engs = [nc.sync, nc.gpsimd]

            def load_xT(b):
                xT = big.tile([128, FC, S], BF16, tag="xT", name="xT")
                for st in range(NS):
                    xn = stage.tile([128, F], BF16, tag="xn", name="xn")
                    nc.scalar.dma_start(out=xn[:ST, :], in_=xr[b, st * ST:(st + 1) * ST, :])
                    slot = pwork.tile([128, 2, 512], F32, tag="proj", name="slot")
                    ptb = slot.bitcast(BF16)  # [128, 2, 1024]
                    for c in range(FC):
                        nc.tensor.transpose(
                            ptb[:, 0, c * ST:(c + 1) * ST],
                            xn[:ST, c * 128:(c + 1) * 128],
                            ident_b[:ST, :ST])
                    nc.vector.tensor_copy(
                        xT[:, :, st * ST:(st + 1) * ST],
                        ptb[:, 0, :FC * ST].rearrange("p (c s) -> p c s", c=FC))
                return xT

            # first head-pair q/k/v weight slices, then batch-0 x, then the rest
            qcount = 0
            for m in range(3):
                eng = dma_engs[qcount % 2]
                qcount += 1
                eng.dma_start(out=wsb[m][:, :, 0:128], in_=ws[m][0])
            xT0 = load_xT(0)
            for co in range(FC):
                for m in range(3):
                    if co == 0:
                        continue
                    eng = dma_engs[qcount % 2]
                    qcount += 1
                    eng.dma_start(
                        out=wsb[m][:, :, co * 128:(co + 1) * 128],
                        in_=ws[m][co])
            for co in range(FC):
                eng = dma_engs[qcount % 2]
                qcount += 1
                eng.dma_start(
                    out=wsb[3][:, :, co * 128:(co + 1) * 128],
                    in_=ws[3][co])

            for b in range(BPC):
                # ---- load x + transpose to feature-major -------------------
                xT = xT0 if b == 0 else load_xT(b)

                if b == 0:
                    _dump("xT", xT[:])

                # ---- fused per-pair projections + attention ---------------
                qT = qkp.tile([128, FC, S], QK_DT, tag="qkT")
                kT = qkp.tile([128, FC, S], QK_DT, tag="qkT")
                vT = vp.tile([128, FC, S], BF16, tag="vT")
                yT = big.tile([128, FC, S], BF16, tag="yT")
                def proj_chunk(m, hp, acc):
                    # 600-token projection of one 128-feature chunk; the h=1
                    # matmul reuses the h=0 stationary weights (no reload).
                    for ci in range(FC):
                        for h in range(2):
                            i = nc.tensor.matmul(
                                acc[:, h, :NH],
                                wsb[m][:, ci, hp * 128:(hp + 1) * 128],
                                xT[:, ci, h * NH:(h + 1) * NH],
                                start=(ci == 0), stop=(ci == FC - 1))
                            if h == 1 and LDW_DEDUP:
                                i.ins.ldweights = False

                for hp in range(FC):
                    # q, k projections (bias add on DVE, f32r out)
                    for m in range(2):
                        dst = (qT, kT)[m]
                        acc = pwork.tile([128, 2, 512], F32, tag="proj")
                        proj_chunk(m, hp, acc)
                        nc.scalar.add(
                            dst[:, hp, :].rearrange("p (h n) -> p h n", h=2),
                            acc[:, :, :NH],
                            add=bsb[m][:, hp:hp + 1])
                    # v projection (bias add on DVE, bf16 out)
                    accv = pwork.tile([128, 2, 512], F32, tag="proj")
                    proj_chunk(2, hp, accv)
                    nc.vector.tensor_scalar_add(
                        vT[:, hp, :].rearrange("p (h n) -> p h n", h=2),
                        in0=accv[:, :, :NH],
                        scalar1=bsb[2][:, hp:hp + 1])

                    # V back to token-major with an appended ones column
                    vn = vnp.tile([128, NS, 2, HD + 1], BF16, tag="vn")
                    nc.gpsimd.memset(vn[:ST, :, :, HD:HD + 1], 1.0)
                    vslot = pwork.tile([128, 2, 512], F32, tag="proj")
                    pv_b = vslot.bitcast(BF16)  # [128, 2, 1024]
                    for t5 in range(NS):
                        nc.tensor.transpose(
                            pv_b[:ST, 0, t5 * 128:(t5 + 1) * 128],
                            vT[:, hp, t5 * ST:(t5 + 1) * ST],
                            ident_b[:, :])
                    nc.vector.tensor_copy(
                        vn[:ST, :, :, 0:HD],
                        pv_b[:ST, 0, :NS * 128].rearrange("p (t g d) -> p t g d",
                                                          t=NS, g=2))
                    if b == 0 and hp == 0:
                        _dump("vn", vn[:])

                    for sh in range(2):
                        pp = ppp.tile([128, NS, 2, NH], BF16, tag="pp")
                        py = pwork.tile([128, 2, 512], F32, tag="proj")
                        for t5 in range(NS):
                            sc = pscore.tile([128, 2, 512], F32, tag="score")
                            for g in range(2):
                                r0 = g * HD
                                nc.tensor.matmul(
                                    sc[:ST, g, :NH],
                                    kT[r0:r0 + HD, hp, t5 * ST:(t5 + 1) * ST],
                                    qT[r0:r0 + HD, hp, sh * NH:(sh + 1) * NH],
                                    start=True, stop=True)
                            nc.scalar.activation(
                                pp[:ST, t5, :, :], sc[:ST, :, :NH],
                                func=mybir.ActivationFunctionType.Exp)
                        for g in range(2):
                            for t5 in range(NS):
                                nc.tensor.matmul(
                                    py[:HD + 1, g, :NH],
                                    vn[:ST, t5, g, :],
                                    pp[:ST, t5, g, :],
                                    start=(t5 == 0), stop=(t5 == NS - 1))
                        if b == 0 and hp == 0 and sh == 0:
                            _dump("pp", pp[:])
                            if KDEBUG:
                                pyc = recp.tile([128, 2, 512], F32, tag="pyc",
                                                bufs=1)
                                nc.vector.tensor_copy(pyc[:], py[:])
                                _dump("py", pyc[:])
                        # softmax normalize: ones-column sums live in
                        # partition HD of each bank
                        srow2 = recp.tile([1, 2, NH], F32, tag="srow2")
                        nc.vector.tensor_copy(srow2[:, :, :], py[HD:HD + 1, :, :NH])
                        rec2 = recp.tile([1, 2, NH], F32, tag="rec2")
                        nc.vector.reciprocal_approx_fast(
                            rec2[:, :, :], srow2[:, :, :])
                        for g in range(2):
                            rb = recp.tile([HD, NH], F32, tag="rb")
                            nc.gpsimd.partition_broadcast(rb[:, :], rec2[0:1, g, :])
                            nc.vector.tensor_mul(
                                yT[g * HD:(g + 1) * HD, hp, sh * NH:(sh + 1) * NH],
                                py[:HD, g, :NH], rb[:, :])
                if b == 0:
                    _dump("qT", qT[:].bitcast(F32) if QK_DT == F32R else qT[:])
                    _dump("kT", kT[:].bitcast(F32) if QK_DT == F32R else kT[:])
                    _dump("vT", vT[:])
                    _dump("yT", yT[:])

                # ---- output projection, token-major -----------------------
                for tb in range(NS):
                    oslot = pwork.tile([128, 2, 512], F32, tag="proj")
                    for ci in range(FC):
                        for half in range(2):
                            i = nc.tensor.matmul(
                                oslot[:ST, half, :FO2],
                                yT[:, ci, tb * ST:(tb + 1) * ST],
                                wsb[3][:, ci, half * FO2:(half + 1) * FO2],
                                start=(ci == 0), stop=(ci == FC - 1))
                            if half == 1 and LDW_DEDUP:
                                i.ins.ldweights = False
                    on = stage.tile([128, F], BF16, tag="on")
                    nc.vector.tensor_add(
                        on[:ST, :].rearrange("p (h n) -> p h n", h=2),
                        oslot[:ST, :, :FO2],
                        bias_o[:ST, :].rearrange("p (h n) -> p h n", h=2))
                    if b == 0 and tb == 0:
                        _dump("on", on[:])
                    nc.sync.dma_start(out=outr[b, tb * ST:(tb + 1) * ST, :],
                                      in_=on[:ST, :])

    nc.finalize()
    return nc


_NC_CACHE = {}


def _get_nc():
    if "nc" not in _NC_CACHE:
        _NC_CACHE["nc"] = _build_nc()
    return _NC_CACHE["nc"]


# ------------------------------------------------------------- host wrapper
def _head_major_perm():
    perm = np.empty(F, dtype=np.int64)
    i = 0
    for h1 in range(H1):
        for h2 in range(H2):
            for h3 in range(H3):
                for x in range(XD):
                    for y in range(YD):
                        for z in range(ZD):
                            a = x * H1 + h1
                            bb = y * H2 + h2
                            cc = z * H3 + h3
                            perm[i] = a * 96 + bb * 12 + cc
                            i += 1
    return perm


def _prep_inputs(inputs):
    perm = _head_major_perm()
    scale = float(HD) ** -0.5

    def kron3(w1, w2, w3):
        return np.kron(w1, np.kron(w2, w3)).astype(np.float32)

    mats = {}
    mats["w0"] = np.ascontiguousarray(
        (kron3(inputs["Wq1"], inputs["Wq2"], inputs["Wq3"])[perm, :] * scale).T)
    mats["b0"] = np.ascontiguousarray(
        inputs["bq"].reshape(F)[perm] * scale).astype(np.float32)
    mats["w1"] = np.ascontiguousarray(
        kron3(inputs["Wk1"], inputs["Wk2"], inputs["Wk3"])[perm, :].T)
    mats["b1"] = np.ascontiguousarray(inputs["bk"].reshape(F)[perm]).astype(np.float32)
    mats["w2"] = np.ascontiguousarray(
        kron3(inputs["Wv1"], inputs["Wv2"], inputs["Wv3"])[perm, :].T)
    mats["b2"] = np.ascontiguousarray(inputs["bv"].reshape(F)[perm]).astype(np.float32)
    mats["w3"] = np.ascontiguousarray(
        kron3(inputs["Wo1"], inputs["Wo2"], inputs["Wo3"])[:, perm].T)
    mats["b3"] = np.ascontiguousarray(inputs["bo"].reshape(F)).astype(np.float32)
    return mats


def _make_in_maps(inputs):
    mats = _prep_inputs(inputs)
    for k in ("w0", "w1", "w2", "w3"):
        mats[k] = np.ascontiguousarray(
            mats[k].reshape(FC, 128, FC, 128).transpose(2, 1, 0, 3)
        ).astype(ml_dtypes.bfloat16)
    x = np.asarray(inputs["x"], dtype=np.float32).reshape(B, S, F)
    x = np.ascontiguousarray(x).astype(ml_dtypes.bfloat16)
    in_maps = []
    for c in range(N_CORES):
        m = {"x": np.ascontiguousarray(x[c * BPC:(c + 1) * BPC])}
        m.update(mats)
        in_maps.append(m)
    return in_maps


def kernel(**inputs) -> np.ndarray:
    nc = _get_nc()
    in_maps = _make_in_maps(inputs)
    res = run_bass_kernel_spmd(nc, in_maps, core_ids=list(range(N_CORES)))
    out = np.concatenate([res.results[c]["out"] for c in range(N_CORES)], axis=0)
    return out.reshape(B, P1, P2, 8, 8, 12).astype(np.float32)


def run_traced(inputs, **kw):
    """test.py helper: returns (output, BassKernelResults) with trace."""
    nc = _get_nc()
    in_maps = _make_in_maps(inputs)
    res = run_bass_kernel_spmd(nc, in_maps, core_ids=list(range(N_CORES)), **kw)
    out = np.concatenate([res.results[c]["out"] for c in range(N_CORES)], axis=0)
    return out.reshape(B, P1, P2, 8, 8, 12).astype(np.float32), res


# revision 41
# speedup vs baseline: 1.0620x; 1.0054x over previous
"""Trainium2 Bass kernel for the Tucker-factorized (TLE) multi-head attention.

Strategy (v2)
-------------
Data-parallel over batch: 16 batches / 8 cores = 2 batches per core; every
core runs the full per-batch pipeline (no collectives needed).

Host-side prep: the three per-mode factor matrices of each projection are
folded into one dense 768x768 Kronecker matrix.  Rows (for q/k/v) are
permuted to *head-major* order (h1,h2,h3,x,y,z) so each of the 12 heads
occupies a contiguous 64-partition block -- two heads per 128-partition
chunk.  The softmax scale 1/8 is folded into the q matrix/bias.  The o
matrix gets the inverse permutation on its rows.

v2 changes vs v1 (394us baseline):
  * Weights + xT + vT/vn/pp/yT in bfloat16: FWL (fast weight load) halves
    LDWEIGHTS on the projection matmuls, transposes run 1 cyc/row, DVE
    copies hit the 2x 16-bit mode, and PE power drops (the v1 trace showed
    the PE SW-throttled to 1.2 GHz for 53% of the run).
  * q/k + scores stay float32r for exp-input precision.
  * Score matmuls for the two heads of a chunk are issued adjacently: head0
    uses PE rows 0-63, head1 rows 64-127 (tile_position auto-derived from
    base_partition) so the two matmuls run concurrently in the array.
  * exp is batched over both heads (one ACT instruction per (sh, t5) pair
    reading both score banks) to amortize ScalarE access latency.
  * q/k bias adds moved to ScalarE (activation Identity+bias); softmax
    reciprocal reads PSUM directly (no staging copy).
  * Output projection computed token-major (out = yT.T @ Wo) which kills
    the final transpose pass entirely; bias added via a partition-broadcast
    bias tile on DVE.
"""

import numpy as np
import ml_dtypes

import concourse.bass as bass
import concourse.tile as tile
from concourse import bacc, mybir
from concourse.bass_utils import run_bass_kernel_spmd
from concourse.masks import make_identity

# ---------------------------------------------------------------- constants
N_CORES = 8
B = 16
BPC = B // N_CORES          # batches per core
P1, P2 = 25, 24
S = P1 * P2                 # 600 tokens
F = 768                     # flattened feature dim
FC = 6                      # feature chunks of 128
ST = 120                    # token tile
NS = S // ST                # 5 token tiles
NH = 300                    # half of the token axis
H1, H2, H3 = 2, 2, 3
XD = YD = ZD = 4
NHEADS = H1 * H2 * H3       # 12
HD = 64
F32 = mybir.dt.float32
BF16 = mybir.dt.bfloat16
F32R = mybir.dt.float32r
FO2 = F // 2                # 384, half of output-feature axis

import os
KDEBUG = os.environ.get("KDEBUG") == "1"
LDW_DEDUP = os.environ.get("LDW_DEDUP", "0") == "1"
QK_DT = BF16 if os.environ.get("QK_BF16", "0") == "1" else F32R


# ---------------------------------------------------------------- device IR
def _build_nc():
    nc = bacc.Bacc("TRN2", target_bir_lowering=False, debug=False)
    xr = nc.declare_dram_parameter("x", [BPC, S, F], BF16, isOutput=False)
    ws = [nc.declare_dram_parameter(f"w{m}", [FC, 128, FC, 128], BF16, isOutput=False)
          for m in range(4)]
    bs = [nc.declare_dram_parameter(f"b{m}", [F], F32, isOutput=False)
          for m in range(4)]
    outr = nc.declare_dram_parameter("out", [BPC, S, F], BF16, isOutput=True)
    dbg = {}
    if KDEBUG:
        for nm in ("xT", "vT", "yT"):
            dbg[nm] = nc.declare_dram_parameter(f"dbg_{nm}", [128, FC, S], BF16,
                                                isOutput=True)
        for nm in ("qT", "kT"):
            dbg[nm] = nc.declare_dram_parameter(
                f"dbg_{nm}", [128, FC, S],
                F32 if QK_DT == F32R else QK_DT, isOutput=True)
        dbg["vn"] = nc.declare_dram_parameter("dbg_vn", [128, NS, 2, HD + 1], BF16,
                                              isOutput=True)
        dbg["pp"] = nc.declare_dram_parameter("dbg_pp", [128, NS, 2, NH], BF16,
                                              isOutput=True)
        dbg["on"] = nc.declare_dram_parameter("dbg_on", [128, F], BF16,
                                              isOutput=True)
        dbg["py"] = nc.declare_dram_parameter("dbg_py", [128, 2, 512], F32,
                                              isOutput=True)
        dbg["rec2"] = nc.declare_dram_parameter("dbg_rec2", [1, 2, NH], F32,
                                                isOutput=True)
        dbg["rb"] = nc.declare_dram_parameter("dbg_rb", [HD, NH], F32,
                                              isOutput=True)

    def _dump(nm, ap):
        if KDEBUG:
            if ap.dtype == F32R:
                ap = ap.bitcast(F32)
            nc.sync.dma_start(out=dbg[nm][:], in_=ap)

    with tile.TileContext(nc) as tc:
        from contextlib import ExitStack
        with ExitStack() as ctx:
            const = ctx.enter_context(tc.tile_pool(name="const", bufs=1))
            big = ctx.enter_context(tc.tile_pool(name="big", bufs=2))
            qkp = ctx.enter_context(tc.tile_pool(name="qkp", bufs=3))
            vp = ctx.enter_context(tc.tile_pool(name="vp", bufs=3))
            stage = ctx.enter_context(tc.tile_pool(name="stage", bufs=4))
            vnp = ctx.enter_context(tc.tile_pool(name="vnp", bufs=4))
            ppp = ctx.enter_context(tc.tile_pool(name="ppp", bufs=4))
            recp = ctx.enter_context(tc.tile_pool(name="recp", bufs=4))
            # PSUM: 2 tags x 2 bufs x 2 banks = 8 banks total
            pwork = ctx.enter_context(tc.tile_pool(name="pwork", bufs=2, space="PSUM"))
            pscore = ctx.enter_context(tc.tile_pool(name="pscore", bufs=2, space="PSUM"))

            ident = const.tile([128, 128], F32, tag="ident")
            make_identity(nc, ident[:])
            ident_b = const.tile([128, 128], BF16, tag="identb")
            nc.vector.tensor_copy(ident_b[:], ident[:])
            ident_r = const.tile([128, 128], F32R, tag="identr")
            nc.vector.tensor_copy(ident_r[:], ident[:])

            # weights [128, 6(fi-chunk), 768(fo)] bf16 and biases.
            # Loaded in per-head-pair column slices, q/k/v slices first, on
            # two DMA queues, so pair-0 attention starts early.
            wsb = []
            for m in range(4):
                w = const.tile([128, FC, F], BF16, tag=f"w{m}")
                wsb.append(w)
            bsb = []
            for m in range(3):
                b = const.tile([128, FC], F32, tag=f"b{m}")
                nc.gpsimd.dma_start(out=b[:], in_=bs[m].rearrange("(c p) -> p c", p=128))
                bsb.append(b)
            # output bias, broadcast along partitions (free axis = fo)
            bo_row = const.tile([1, F], F32, tag="bo_row")
            nc.gpsimd.dma_start(out=bo_row[:, :], in_=bs[3].rearrange("(p f) -> p f", p=1))
            bias_o = const.tile([128, F], F32, tag="bias_o")
            nc.gpsimd.partition_broadcast(bias_o[:, :], bo_row[0:1, :])

            dma_engs = [nc.sync, nc.gpsimd]

            def load_xT(b):
                xT = big.tile([128, FC, S], BF16, tag="xT", name="xT")
                for st in range(NS):
                    xn = stage.tile([128, F], BF16, tag="xn", name="xn")
                    nc.scalar.dma_start(out=xn[:ST, :], in_=xr[b, st * ST:(st + 1) * ST, :])
                    slot = pwork.tile([128, 2, 512], F32, tag="proj", name="slot")
                    ptb = slot.bitcast(BF16)  # [128, 2, 1024]
                    for c in range(FC):
                        nc.tensor.transpose(
                            ptb[:, 0, c * ST:(c + 1) * ST],
                            xn[:ST, c * 128:(c + 1) * 128],
                            ident_b[:ST, :ST])
                    nc.vector.tensor_copy(
                        xT[:, :, st * ST:(st + 1) * ST],
                        ptb[:, 0, :FC * ST].rearrange("p (c s) -> p c s", c=FC))
                return xT

            # first head-pair q/k/v weight slices, then batch-0 x, then the rest
            qcount = 0
            for m in range(3):
                eng = dma_engs[qcount % 2]
                qcount += 1
                eng.dma_start(out=wsb[m][:, :, 0:128], in_=ws[m][0])
            xT0 = load_xT(0)
            for co in range(FC):
                for m in range(3):
                    if co == 0:
                        continue
                    eng = dma_engs[qcount % 2]
                    qcount += 1
                    eng.dma_start(
                        out=wsb[m][:, :, co * 128:(co + 1) * 128],
                        in_=ws[m][co])
            for co in range(FC):
                eng = dma_engs[qcount % 2]
                qcount += 1
                eng.dma_start(
                    out=wsb[3][:, :, co * 128:(co + 1) * 128],
                    in_=ws[3][co])

            for b in range(BPC):
                # ---- load x + transpose to feature-major -------------------
                xT = xT0 if b == 0 else load_xT(b)

                if b == 0:
                    _dump("xT", xT[:])

                # ---- fused per-pair projections + attention ---------------
                qT = qkp.tile([128, FC, S], QK_DT, tag="qkT")
                kT = qkp.tile([128, FC, S], QK_DT, tag="qkT")
                vT = vp.tile([128, FC, S], BF16, tag="vT")
                yT = big.tile([128, FC, S], BF16, tag="yT")
                def proj_chunk(m, hp, acc):
                    # 600-token projection of one 128-feature chunk; the h=1
                    # matmul reuses the h=0 stationary weights (no reload).
                    for ci in range(FC):
                        for h in range(2):
                            i = nc.tensor.matmul(
                                acc[:, h, :NH],
                                wsb[m][:, ci, hp * 128:(hp + 1) * 128],
                                xT[:, ci, h * NH:(h + 1) * NH],
                                start=(ci == 0), stop=(ci == FC - 1))
                            if h == 1 and LDW_DEDUP:
                                i.ins.ldweights = False

                for hp in range(FC):
                    # q, k projections (bias add on DVE, f32r out)
                    for m in range(2):
                        dst = (qT, kT)[m]
                        acc = pwork.tile([128, 2, 512], F32, tag="proj")
                        proj_chunk(m, hp, acc)
                        nc.scalar.add(
                            dst[:, hp, :].rearrange("p (h n) -> p h n", h=2),
                            acc[:, :, :NH],
                            add=bsb[m][:, hp:hp + 1])
                    # v projection (bias add on DVE, bf16 out)
                    accv = pwork.tile([128, 2, 512], F32, tag="proj")
                    proj_chunk(2, hp, accv)
                    nc.vector.tensor_scalar_add(
                        vT[:, hp, :].rearrange("p (h n) -> p h n", h=2),
                        in0=accv[:, :, :NH],
                        scalar1=bsb[2][:, hp:hp + 1])

                    # V back to token-major with an appended ones column
                    vn = vnp.tile([128, NS, 2, HD + 1], BF16, tag="vn")
                    nc.gpsimd.memset(vn[:ST, :, :, HD:HD + 1], 1.0)
                    vslot = pwork.tile([128, 2, 512], F32, tag="proj")
                    pv_b = vslot.bitcast(BF16)  # [128, 2, 1024]
                    for t5 in range(NS):
                        nc.tensor.transpose(
                            pv_b[:ST, 0, t5 * 128:(t5 + 1) * 128],
                            vT[:, hp, t5 * ST:(t5 + 1) * ST],
                            ident_b[:, :])
                    nc.vector.tensor_copy(
                        vn[:ST, :, :, 0:HD],
                        pv_b[:ST, 0, :NS * 128].rearrange("p (t g d) -> p t g d",
                                                          t=NS, g=2))
                    if b == 0 and hp == 0:
                        _dump("vn", vn[:])

                    for sh in range(2):
                        pp = ppp.tile([128, NS, 2, NH], BF16, tag="pp")
                        py = pwork.tile([128, 2, 512], F32, tag="proj")
                        for t5 in range(NS):
                            sc = pscore.tile([128, 2, 512], F32, tag="score")
                            for g in range(2):
                                r0 = g * HD
                                nc.tensor.matmul(
                                    sc[:ST, g, :NH],
                                    kT[r0:r0 + HD, hp, t5 * ST:(t5 + 1) * ST],
                                    qT[r0:r0 + HD, hp, sh * NH:(sh + 1) * NH],
                                    start=True, stop=True)
                            nc.scalar.activation(
                                pp[:ST, t5, :, :], sc[:ST, :, :NH],
                                func=mybir.ActivationFunctionType.Exp)
                        for g in range(2):
                            for t5 in range(NS):
                                nc.tensor.matmul(
                                    py[:HD + 1, g, :NH],
                                    vn[:ST, t5, g, :],
                                    pp[:ST, t5, g, :],
                                    start=(t5 == 0), stop=(t5 == NS - 1))
                        if b == 0 and hp == 0 and sh == 0:
                            _dump("pp", pp[:])
                            if KDEBUG:
                                pyc = recp.tile([128, 2, 512], F32, tag="pyc",
                                                bufs=1)
                                nc.vector.tensor_copy(pyc[:], py[:])
                                _dump("py", pyc[:])
                        # softmax normalize: ones-column sums live in
                        # partition HD of each bank
                        srow2 = recp.tile([1, 2, NH], F32, tag="srow2")
                        nc.vector.tensor_copy(srow2[:, :, :], py[HD:HD + 1, :, :NH])
                        rec2 = recp.tile([1, 2, NH], F32, tag="rec2")
                        nc.vector.reciprocal_approx_fast(
                            rec2[:, :, :], srow2[:, :, :])
                        for g in range(2):
                            rb = recp.tile([HD, NH], F32, tag="rb")
                            nc.gpsimd.partition_broadcast(rb[:, :], rec2[0:1, g, :])
                            nc.vector.tensor_mul(
                                yT[g * HD:(g + 1) * HD, hp, sh * NH:(sh + 1) * NH],
                                py[:HD, g, :NH], rb[:, :])
                if b == 0:
                    _dump("qT", qT[:].bitcast(F32) if QK_DT == F32R else qT[:])
                    _dump("kT", kT[:].bitcast(F32) if QK_DT == F32R else kT[:])
                    _dump("vT", vT[:])
                    _dump("yT", yT[:])

                # ---- output projection, token-major -----------------------
                for tb in range(NS):
                    oslot = pwork.tile([128, 2, 512], F32, tag="proj")
                    for ci in range(FC):
                        for half in range(2):
                            i = nc.tensor.matmul(
                                oslot[:ST, half, :FO2],
                                yT[:, ci, tb * ST:(tb + 1) * ST],
                                wsb[3][:, ci, half * FO2:(half + 1) * FO2],
                                start=(ci == 0), stop=(ci == FC - 1))
                            if half == 1 and LDW_DEDUP:
                                i.ins.ldweights = False
                    on = stage.tile([128, F], BF16, tag="on")
                    nc.vector.tensor_add(
                        on[:ST, :].rearrange("p (h n) -> p h n", h=2),
                        oslot[:ST, :, :FO2],
                        bias_o[:ST, :].rearrange("p (h n) -> p h n", h=2))
                    if b == 0 and tb == 0:
                        _dump("on", on[:])
                    nc.sync.dma_start(out=outr[b, tb * ST:(tb + 1) * ST, :],
                                      in_=on[:ST, :])

    nc.finalize()
    return nc


_NC_CACHE = {}


def _get_nc():
    if "nc" not in _NC_CACHE:
        _NC_CACHE["nc"] = _build_nc()
    return _NC_CACHE["nc"]


# ------------------------------------------------------------- host wrapper
def _head_major_perm():
    perm = np.empty(F, dtype=np.int64)
    i = 0
    for h1 in range(H1):
        for h2 in range(H2):
            for h3 in range(H3):
                for x in range(XD):
                    for y in range(YD):
                        for z in range(ZD):
                            a = x * H1 + h1
                            bb = y * H2 + h2
                            cc = z * H3 + h3
                            perm[i] = a * 96 + bb * 12 + cc
                            i += 1
    return perm


def _prep_inputs(inputs):
    perm = _head_major_perm()
    scale = float(HD) ** -0.5

    def kron3(w1, w2, w3):
        return np.kron(w1, np.kron(w2, w3)).astype(np.float32)

    mats = {}
    mats["w0"] = np.ascontiguousarray(
        (kron3(inputs["Wq1"], inputs["Wq2"], inputs["Wq3"])[perm, :] * scale).T)
    mats["b0"] = np.ascontiguousarray(
        inputs["bq"].reshape(F)[perm] * scale).astype(np.float32)
    mats["w1"] = np.ascontiguousarray(
        kron3(inputs["Wk1"], inputs["Wk2"], inputs["Wk3"])[perm, :].T)
    mats["b1"] = np.ascontiguousarray(inputs["bk"].reshape(F)[perm]).astype(np.float32)
    mats["w2"] = np.ascontiguousarray(
        kron3(inputs["Wv1"], inputs["Wv2"], inputs["Wv3"])[perm, :].T)
    mats["b2"] = np.ascontiguousarray(inputs["bv"].reshape(F)[perm]).astype(np.float32)
    mats["w3"] = np.ascontiguousarray(
        kron3(inputs["Wo1"], inputs["Wo2"], inputs["Wo3"])[:, perm].T)
    mats["b3"] = np.ascontiguousarray(inputs["bo"].reshape(F)).astype(np.float32)
    return mats


def _make_in_maps(inputs):
    mats = _prep_inputs(inputs)
    for k in ("w0", "w1", "w2", "w3"):
        mats[k] = np.ascontiguousarray(
            mats[k].reshape(FC, 128, FC, 128).transpose(2, 1, 0, 3)
        ).astype(ml_dtypes.bfloat16)
    x = np.asarray(inputs["x"], dtype=np.float32).reshape(B, S, F)
    x = np.ascontiguousarray(x).astype(ml_dtypes.bfloat16)
    in_maps = []
    for c in range(N_CORES):
        m = {"x": np.ascontiguousarray(x[c * BPC:(c + 1) * BPC])}
        m.update(mats)
        in_maps.append(m)
    return in_maps


def kernel(**inputs) -> np.ndarray:
    nc = _get_nc()
    in_maps = _make_in_maps(inputs)
    res = run_bass_kernel_spmd(nc, in_maps, core_ids=list(range(N_CORES)))
    out = np.concatenate([res.results[c]["out"] for c in range(N_CORES)], axis=0)
    return out.reshape(B, P1, P2, 8, 8, 12).astype(np.float32)


def run_traced(inputs, **kw):
    """test.py helper: returns (output, BassKernelResults) with trace."""
    nc = _get_nc()
    in_maps = _make_in_maps(inputs)
    res = run_bass_kernel_spmd(nc, in_maps, core_ids=list(range(N_CORES)), **kw)
    out = np.concatenate([res.results[c]["out"] for c in range(N_CORES)], axis=0)
    return out.reshape(B, P1, P2, 8, 8, 12).astype(np.float32), res


# revision 48
# speedup vs baseline: 1.0693x; 1.0069x over previous
"""Trainium2 Bass kernel for the Tucker-factorized (TLE) multi-head attention.

Strategy (v2)
-------------
Data-parallel over batch: 16 batches / 8 cores = 2 batches per core; every
core runs the full per-batch pipeline (no collectives needed).

Host-side prep: the three per-mode factor matrices of each projection are
folded into one dense 768x768 Kronecker matrix.  Rows (for q/k/v) are
permuted to *head-major* order (h1,h2,h3,x,y,z) so each of the 12 heads
occupies a contiguous 64-partition block -- two heads per 128-partition
chunk.  The softmax scale 1/8 is folded into the q matrix/bias.  The o
matrix gets the inverse permutation on its rows.

v2 changes vs v1 (394us baseline):
  * Weights + xT + vT/vn/pp/yT in bfloat16: FWL (fast weight load) halves
    LDWEIGHTS on the projection matmuls, transposes run 1 cyc/row, DVE
    copies hit the 2x 16-bit mode, and PE power drops (the v1 trace showed
    the PE SW-throttled to 1.2 GHz for 53% of the run).
  * q/k + scores stay float32r for exp-input precision.
  * Score matmuls for the two heads of a chunk are issued adjacently: head0
    uses PE rows 0-63, head1 rows 64-127 (tile_position auto-derived from
    base_partition) so the two matmuls run concurrently in the array.
  * exp is batched over both heads (one ACT instruction per (sh, t5) pair
    reading both score banks) to amortize ScalarE access latency.
  * q/k bias adds moved to ScalarE (activation Identity+bias); softmax
    reciprocal reads PSUM directly (no staging copy).
  * Output projection computed token-major (out = yT.T @ Wo) which kills
    the final transpose pass entirely; bias added via a partition-broadcast
    bias tile on DVE.
"""

import numpy as np
import ml_dtypes

import concourse.bass as bass
import concourse.tile as tile
from concourse import bacc, mybir
from concourse.bass_utils import run_bass_kernel_spmd
from concourse.masks import make_identity

# ---------------------------------------------------------------- constants
N_CORES = 8
B = 16
BPC = B // N_CORES          # batches per core
P1, P2 = 25, 24
S = P1 * P2                 # 600 tokens
F = 768                     # flattened feature dim
FC = 6                      # feature chunks of 128
ST = 120                    # token tile
NS = S // ST                # 5 token tiles
NH = 300                    # half of the token axis
H1, H2, H3 = 2, 2, 3
XD = YD = ZD = 4
NHEADS = H1 * H2 * H3       # 12
HD = 64
F32 = mybir.dt.float32
BF16 = mybir.dt.bfloat16
F32R = mybir.dt.float32r
FP8 = mybir.dt.float8e4
FO2 = F // 2                # 384, half of output-feature axis

import os
KDEBUG = os.environ.get("KDEBUG") == "1"
LDW_DEDUP = os.environ.get("LDW_DEDUP", "0") == "1"
QK_DT = BF16 if os.environ.get("QK_BF16", "0") == "1" else F32R


# ---------------------------------------------------------------- device IR
def _build_nc():
    nc = bacc.Bacc("TRN2", target_bir_lowering=False, debug=False)
    xr = nc.declare_dram_parameter("x", [BPC, S, F], BF16, isOutput=False)
    ws = [nc.declare_dram_parameter(f"w{m}", [FC, 128, FC, 128], BF16, isOutput=False)
          for m in range(4)]
    bs = [nc.declare_dram_parameter(f"b{m}", [F], F32, isOutput=False)
          for m in range(4)]
    outr = nc.declare_dram_parameter("out", [BPC, S, F], BF16, isOutput=True)
    dbg = {}
    if KDEBUG:
        for nm in ("xT", "vT"):
            dbg[nm] = nc.declare_dram_parameter(f"dbg_{nm}", [128, FC, S], BF16,
                                                isOutput=True)
        dbg["yT"] = nc.declare_dram_parameter("dbg_yT", [128, FC, S], BF16,
                                              isOutput=True)
        for nm in ("qT", "kT"):
            dbg[nm] = nc.declare_dram_parameter(
                f"dbg_{nm}", [128, FC, S],
                F32 if QK_DT == F32R else QK_DT, isOutput=True)
        dbg["vn"] = nc.declare_dram_parameter("dbg_vn", [128, NS, 2, HD + 1], BF16,
                                              isOutput=True)
        dbg["pp"] = nc.declare_dram_parameter("dbg_pp", [128, NS, 2, NH], BF16,
                                              isOutput=True)
        dbg["on"] = nc.declare_dram_parameter("dbg_on", [128, F], BF16,
                                              isOutput=True)
        dbg["py"] = nc.declare_dram_parameter("dbg_py", [128, 2, 512], F32,
                                              isOutput=True)
        dbg["rec2"] = nc.declare_dram_parameter("dbg_rec2", [1, 2, NH], F32,
                                                isOutput=True)
        dbg["rb"] = nc.declare_dram_parameter("dbg_rb", [HD, NH], F32,
                                              isOutput=True)

    def _dump(nm, ap):
        if KDEBUG:
            if ap.dtype == F32R:
                ap = ap.bitcast(F32)
            nc.sync.dma_start(out=dbg[nm][:], in_=ap)

    with tile.TileContext(nc) as tc:
        from contextlib import ExitStack
        with ExitStack() as ctx:
            const = ctx.enter_context(tc.tile_pool(name="const", bufs=1))
            big = ctx.enter_context(tc.tile_pool(name="big", bufs=2))
            qkp = ctx.enter_context(tc.tile_pool(name="qkp", bufs=3))
            vp = ctx.enter_context(tc.tile_pool(name="vp", bufs=3))
            stage = ctx.enter_context(tc.tile_pool(name="stage", bufs=4))
            vnp = ctx.enter_context(tc.tile_pool(name="vnp", bufs=4))
            ppp = ctx.enter_context(tc.tile_pool(name="ppp", bufs=4))
            recp = ctx.enter_context(tc.tile_pool(name="recp", bufs=4))
            # PSUM: 2 tags x 2 bufs x 2 banks = 8 banks total
            pwork = ctx.enter_context(tc.tile_pool(name="pwork", bufs=2, space="PSUM"))
            pscore = ctx.enter_context(tc.tile_pool(name="pscore", bufs=2, space="PSUM"))

            ident = const.tile([128, 128], F32, tag="ident")
            make_identity(nc, ident[:])
            ident_b = const.tile([128, 128], BF16, tag="identb")
            nc.vector.tensor_copy(ident_b[:], ident[:])
            ident_r = const.tile([128, 128], F32R, tag="identr")
            nc.vector.tensor_copy(ident_r[:], ident[:])

            # weights [128, 6(fi-chunk), 768(fo)] bf16 and biases.
            # Loaded in per-head-pair column slices, q/k/v slices first, on
            # two DMA queues, so pair-0 attention starts early.
            wsb = []
            for m in range(4):
                w = const.tile([128, FC, F], BF16, tag=f"w{m}")
                wsb.append(w)
            bsb = []
            for m in range(3):
                b = const.tile([128, FC], F32, tag=f"b{m}")
                nc.gpsimd.dma_start(out=b[:], in_=bs[m].rearrange("(c p) -> p c", p=128))
                bsb.append(b)
            # output bias, broadcast along partitions (free axis = fo)
            bo_row = const.tile([1, F], F32, tag="bo_row")
            nc.gpsimd.dma_start(out=bo_row[:, :], in_=bs[3].rearrange("(p f) -> p f", p=1))
            bias_o = const.tile([128, F], F32, tag="bias_o")
            nc.gpsimd.partition_broadcast(bias_o[:, :], bo_row[0:1, :])

            dma_engs = [nc.sync, nc.gpsimd]

            def load_xT(b):
                xT = big.tile([128, FC, S], BF16, tag="xT", name="xT")
                for st in range(NS):
                    xn = stage.tile([128, F], BF16, tag="xn", name="xn")
                    nc.scalar.dma_start(out=xn[:ST, :], in_=xr[b, st * ST:(st + 1) * ST, :])
                    slot = pwork.tile([128, 2, 512], F32, tag="proj", name="slot")
                    ptb = slot.bitcast(BF16)  # [128, 2, 1024]
                    for c in range(FC):
                        nc.tensor.transpose(
                            ptb[:, 0, c * ST:(c + 1) * ST],
                            xn[:ST, c * 128:(c + 1) * 128],
                            ident_b[:ST, :ST])
                    nc.vector.tensor_copy(
                        xT[:, :, st * ST:(st + 1) * ST],
                        ptb[:, 0, :FC * ST].rearrange("p (c s) -> p c s", c=FC))
                return xT

            # first head-pair q/k/v weight slices, then batch-0 x, then the rest
            def dma_w(m, co):
                eng = dma_engs[dma_w.q % 2]
                dma_w.q += 1
                eng.dma_start(out=wsb[m][:, :, co * 128:(co + 1) * 128],
                              in_=ws[m][co])
            dma_w.q = 0
            for m in range(3):
                dma_w(m, 0)
            xT0 = load_xT(0)
            for co in range(FC):
                for m in range(3):
                    if co == 0:
                        continue
                    dma_w(m, co)
            for co in range(FC):
                dma_w(3, co)

            for b in range(BPC):
                # ---- load x + transpose to feature-major -------------------
                xT = xT0 if b == 0 else load_xT(b)

                if b == 0:
                    _dump("xT", xT[:])

                # ---- fused per-pair projections + attention ---------------
                qT = qkp.tile([128, FC, S], QK_DT, tag="qkT")
                kT = qkp.tile([128, FC, S], QK_DT, tag="qkT")
                vT = vp.tile([128, FC, S], BF16, tag="vT")
                yT = big.tile([128, FC, S], BF16, tag="yT")
                def proj_chunk(m, hp, acc):
                    # 600-token projection of one 128-feature chunk; the h=1
                    # matmul reuses the h=0 stationary weights (no reload).
                    for ci in range(FC):
                        for h in range(2):
                            i = nc.tensor.matmul(
                                acc[:, h, :NH],
                                wsb[m][:, ci, hp * 128:(hp + 1) * 128],
                                xT[:, ci, h * NH:(h + 1) * NH],
                                start=(ci == 0), stop=(ci == FC - 1))
                            if h == 1 and LDW_DEDUP:
                                i.ins.ldweights = False

                for hp in range(FC):
                    # q, k projections (bias add on DVE, f32r out)
                    for m in range(2):
                        dst = (qT, kT)[m]
                        acc = pwork.tile([128, 2, 512], F32, tag="proj")
                        proj_chunk(m, hp, acc)
                        nc.scalar.add(
                            dst[:, hp, :].rearrange("p (h n) -> p h n", h=2),
                            acc[:, :, :NH],
                            add=bsb[m][:, hp:hp + 1])
                    # v projection (bias add on DVE, bf16 out)
                    accv = pwork.tile([128, 2, 512], F32, tag="proj")
                    proj_chunk(2, hp, accv)
                    nc.vector.tensor_scalar_add(
                        vT[:, hp, :].rearrange("p (h n) -> p h n", h=2),
                        in0=accv[:, :, :NH],
                        scalar1=bsb[2][:, hp:hp + 1])

                    # V back to token-major with an appended ones column
                    vn = vnp.tile([128, NS, 2, HD + 1], BF16, tag="vn")
                    nc.gpsimd.memset(vn[:ST, :, :, HD:HD + 1], 1.0)
                    vslot = pwork.tile([128, 2, 512], F32, tag="proj")
                    pv_b = vslot.bitcast(BF16)  # [128, 2, 1024]
                    for t5 in range(NS):
                        nc.tensor.transpose(
                            pv_b[:ST, 0, t5 * 128:(t5 + 1) * 128],
                            vT[:, hp, t5 * ST:(t5 + 1) * ST],
                            ident_b[:, :])
                    nc.vector.tensor_copy(
                        vn[:ST, :, :, 0:HD],
                        pv_b[:ST, 0, :NS * 128].rearrange("p (t g d) -> p t g d",
                                                          t=NS, g=2))
                    if b == 0 and hp == 0:
                        _dump("vn", vn[:])

                    for sh in range(2):
                        pp = ppp.tile([128, NS, 2, NH], BF16, tag="pp")
                        py = pwork.tile([128, 2, 512], F32, tag="proj")
                        for t5 in range(NS):
                            sc = pscore.tile([128, 2, 512], F32, tag="score")
                            for g in range(2):
                                r0 = g * HD
                                nc.tensor.matmul(
                                    sc[:ST, g, :NH],
                                    kT[r0:r0 + HD, hp, t5 * ST:(t5 + 1) * ST],
                                    qT[r0:r0 + HD, hp, sh * NH:(sh + 1) * NH],
                                    start=True, stop=True)
                            nc.scalar.activation(
                                pp[:ST, t5, :, :], sc[:ST, :, :NH],
                                func=mybir.ActivationFunctionType.Exp)
                        for g in range(2):
                            for t5 in range(NS):
                                nc.tensor.matmul(
                                    py[:HD + 1, g, :NH],
                                    vn[:ST, t5, g, :],
                                    pp[:ST, t5, g, :],
                                    start=(t5 == 0), stop=(t5 == NS - 1))
                        if b == 0 and hp == 0 and sh == 0:
                            _dump("pp", pp[:])
                            if KDEBUG:
                                pyc = recp.tile([128, 2, 512], F32, tag="pyc",
                                                bufs=1)
                                nc.vector.tensor_copy(pyc[:], py[:])
                                _dump("py", pyc[:])
                        # softmax normalize: ones-column sums live in
                        # partition HD of each bank
                        srow2 = recp.tile([1, 2, NH], F32, tag="srow2")
                        nc.vector.tensor_copy(srow2[:, :, :], py[HD:HD + 1, :, :NH])
                        rec2 = recp.tile([1, 2, NH], F32, tag="rec2")
                        nc.vector.reciprocal_approx_fast(
                            rec2[:, :, :], srow2[:, :, :])
                        for g in range(2):
                            rb = recp.tile([HD, NH], F32, tag="rb")
                            nc.gpsimd.partition_broadcast(rb[:, :], rec2[0:1, g, :])
                            nc.vector.tensor_mul(
                                yT[g * HD:(g + 1) * HD, hp, sh * NH:(sh + 1) * NH],
                                py[:HD, g, :NH], rb[:, :])
                if b == 0:
                    _dump("qT", qT[:].bitcast(F32) if QK_DT == F32R else qT[:])
                    _dump("kT", kT[:].bitcast(F32) if QK_DT == F32R else kT[:])
                    _dump("vT", vT[:])
                    _dump("yT", yT[:])

                # ---- output projection, token-major -----------------------
                for tb in range(NS):
                    oslot = pwork.tile([128, 2, 512], F32, tag="proj")
                    for ci in range(FC):
                        for half in range(2):
                            nc.tensor.matmul(
                                oslot[:ST, half, :FO2],
                                yT[:, ci, tb * ST:(tb + 1) * ST],
                                wsb[3][:, ci, half * FO2:(half + 1) * FO2],
                                start=(ci == 0), stop=(ci == FC - 1))
                    on = stage.tile([128, F], BF16, tag="on")
                    nc.vector.tensor_add(
                        on[:ST, :].rearrange("p (h n) -> p h n", h=2),
                        oslot[:ST, :, :FO2],
                        bias_o[:ST, :].rearrange("p (h n) -> p h n", h=2))
                    if b == 0 and tb == 0:
                        _dump("on", on[:])
                    nc.sync.dma_start(out=outr[b, tb * ST:(tb + 1) * ST, :],
                                      in_=on[:ST, :])

    nc.finalize()
    return nc


_NC_CACHE = {}


def _get_nc():
    if "nc" not in _NC_CACHE:
        _NC_CACHE["nc"] = _build_nc()
    return _NC_CACHE["nc"]


# ------------------------------------------------------------- host wrapper
def _head_major_perm():
    perm = np.empty(F, dtype=np.int64)
    i = 0
    for h1 in range(H1):
        for h2 in range(H2):
            for h3 in range(H3):
                for x in range(XD):
                    for y in range(YD):
                        for z in range(ZD):
                            a = x * H1 + h1
                            bb = y * H2 + h2
                            cc = z * H3 + h3
                            perm[i] = a * 96 + bb * 12 + cc
                            i += 1
    return perm


def _prep_inputs(inputs):
    perm = _head_major_perm()
    scale = float(HD) ** -0.5

    def kron3(w1, w2, w3):
        return np.kron(w1, np.kron(w2, w3)).astype(np.float32)

    mats = {}
    mats["w0"] = np.ascontiguousarray(
        (kron3(inputs["Wq1"], inputs["Wq2"], inputs["Wq3"])[perm, :] * scale).T)
    mats["b0"] = np.ascontiguousarray(
        inputs["bq"].reshape(F)[perm] * scale).astype(np.float32)
    mats["w1"] = np.ascontiguousarray(
        kron3(inputs["Wk1"], inputs["Wk2"], inputs["Wk3"])[perm, :].T)
    mats["b1"] = np.ascontiguousarray(inputs["bk"].reshape(F)[perm]).astype(np.float32)
    mats["w2"] = np.ascontiguousarray(
        kron3(inputs["Wv1"], inputs["Wv2"], inputs["Wv3"])[perm, :].T)
    mats["b2"] = np.ascontiguousarray(inputs["bv"].reshape(F)[perm]).astype(np.float32)
    mats["w3"] = np.ascontiguousarray(
        kron3(inputs["Wo1"], inputs["Wo2"], inputs["Wo3"])[:, perm].T)
    mats["b3"] = np.ascontiguousarray(inputs["bo"].reshape(F)).astype(np.float32)
    return mats


def _make_in_maps(inputs):
    mats = _prep_inputs(inputs)
    for k in ("w0", "w1"):
        mats[k] = np.ascontiguousarray(
            mats[k].reshape(FC, 128, FC, 128).transpose(2, 1, 0, 3)
        ).astype(ml_dtypes.bfloat16)
    for k in ("w2", "w3"):
        mats[k] = np.ascontiguousarray(
            mats[k].reshape(FC, 128, FC, 128).transpose(2, 1, 0, 3)
        ).astype(ml_dtypes.bfloat16)
    x = np.asarray(inputs["x"], dtype=np.float32).reshape(B, S, F)
    x = np.ascontiguousarray(x).astype(ml_dtypes.bfloat16)
    in_maps = []
    for c in range(N_CORES):
        m = {"x": np.ascontiguousarray(x[c * BPC:(c + 1) * BPC])}
        m.update(mats)
        in_maps.append(m)
    return in_maps


def kernel(**inputs) -> np.ndarray:
    nc = _get_nc()
    in_maps = _make_in_maps(inputs)
    res = run_bass_kernel_spmd(nc, in_maps, core_ids=list(range(N_CORES)))
    out = np.concatenate([res.results[c]["out"] for c in range(N_CORES)], axis=0)
    return out.reshape(B, P1, P2, 8, 8, 12).astype(np.float32)


def run_traced(inputs, **kw):
    """test.py helper: returns (output, BassKernelResults) with trace."""
    nc = _get_nc()
    in_maps = _make_in_maps(inputs)
    res = run_bass_kernel_spmd(nc, in_maps, core_ids=list(range(N_CORES)), **kw)
    out = np.concatenate([res.results[c]["out"] for c in range(N_CORES)], axis=0)
    return out.reshape(B, P1, P2, 8, 8, 12).astype(np.float32), res
